# revision 19
# baseline (speedup 1.0000x reference)
"""CTC loss (nn_CTC_28819230556189) on 8 Trainium2 NeuronCores via Bass/Tile.

Data-parallel over batch (4 examples/core). Per core:

  Phase 1 (PE + Act):  logits = hpad @ W.T in fp8 DoubleRow (K=256/matmul);
    lse side:  exp(logit - C) accumulated over V -> lsum[t]; ln(lsum) columns
               collected and partition-reduced ONCE at the end via a
               ones-matmul -> Sum_t ln lsum per example (llacc).
    glog side: the extended label sequence has only 101 distinct tokens per
               example (blank + 100 labels), so glog^T is computed as a
               [101, t] matmul; p~ = exp(glog + D) in bf16 is DMA-transposed
               into psweep[example, row, t] (row 0 = blank, row 1+j = label j).
    The -lse term is NOT folded into p~: every CTC path takes exactly one
    emission per frame, so ll = ln(sum B~) - Sum_t lse_t - T*D, with
    lse_t = ln lsum_t + C.

  Phase 2 (DVE): CTC forward DP as a label-sweep of pure affine scans
    (tensor_tensor_scan, fp32 internal state).  With E_j = alpha[blank 2j],
    O_j = alpha[label 2j+1], and F_j = E_j + allow_j * O_{j-1}:
        F_j(t) = q(t)*F_j(t-1) + O_{j-1}(t)        (scan: mult, add)
        O_j(t) = (O_j(t-1) + F_j(t-1)) * p_j(t)    (scan: add, mult)
    and the likelihood is simply F_L(T-1) = alpha_T[S-1] + alpha_T[S-2].
    201 scans total, no elementwise adds; each scan runs only over the
    frame window from which terminal states remain reachable (~402 of 500).
    Repeated labels (skip disallowed) get a rare blended-input fixup.

  Loss partials summed on host (no collectives needed).
"""

import numpy as np

import concourse.bass as bass
import concourse.bacc as bacc
import concourse.tile as tile
import concourse.mybir as mybir
from concourse.bass_utils import run_bass_kernel_spmd

BF16 = mybir.dt.bfloat16
F32 = mybir.dt.float32
FP8 = mybir.dt.float8e4
AF = mybir.ActivationFunctionType
ALU = mybir.AluOpType
AX = mybir.AxisListType
DR = mybir.MatmulPerfMode.DoubleRow

# Problem shapes (hardcoded per spec nn_CTC_28819230556189)
B, T, E, V, L = 32, 500, 1024, 5000, 100
S = 2 * L + 1           # 201 extended labels
NCORE = 8
BPC = B // NCORE        # 4 examples per core
NPAIR = E // 256        # 4 double-row K-pairs (256 contraction each)
TC = 125                # time chunk
NCHUNK = T // TC        # 4
VC = 500                # v-chunk width (one PSUM bank in f32)
NV = V // VC            # 10
C_SHIFT = 4.0           # logsumexp constant shift (logits ~ N(0,1))
D_SHIFT = -1.1          # p~ = exp(glog + D); keeps ln(sum B~) drift ~ 0
NTOK = L + 1            # distinct p~ rows per example (blank + labels)
TCP = 128               # hp stationary pair-stride (16B-aligned pad of TC)
RP = 112                # wext stationary pair-stride (16B-aligned pad of NTOK)

_cache = {}


def _hi(s):
    """Last frame (inclusive) from which extended state s can still reach a
    terminal state ({S-2, S-1}) by frame T-1."""
    need = max(0, (S - 2) - s)
    return min(T - 1, T - 1 - (need + 1) // 2)


def _build_nc(masked_j):
    """masked_j: sorted tuple of label indices j (1..L-1) where some example
    has ys[j] == ys[j-1] (skip transition disallowed) -> those F_j scans get
    a per-example blended input; all other j use O_{j-1} directly."""
    nc = bacc.Bacc("TRN2", target_bir_lowering=False, debug=False,
                   enable_asserts=False)

    for val in (-C_SHIFT, D_SHIFT):
        cth = nc.alloc_sbuf_tensor(f"const-f32-{val}", [128, 1], F32)
        nc.gpsimd.memset(cth.ap(), val)
        nc.const_aps.aps[(F32, val)] = cth.ap()
    nc.all_engine_barrier()

    hpt_d = nc.dram_tensor("hpt", [BPC, NCHUNK, 128, NPAIR * 2 * TCP],
                           FP8, kind="ExternalInput")
    wtt_d = nc.dram_tensor("wtt", [NPAIR, 2, 128, V], FP8,
                           kind="ExternalInput")
    wxt_d = nc.dram_tensor("wxt", [BPC, NPAIR, 2, 128, NTOK], FP8,
                           kind="ExternalInput")
    # m2[:, 0, j] = allow skip into label j; m2[:, 1, j] = 1 - that
    m2_d = nc.dram_tensor("m2", [BPC, 2, NTOK], F32, kind="ExternalInput")
    out_d = nc.dram_tensor("out", [1, 1], F32, kind="ExternalOutput")

    with tile.TileContext(nc) as tc:
      with tc.tile_pool(name="persist", bufs=1) as pers:
        def ptile(shape, dtype, nm):
            return pers.tile(shape, dtype, tag=nm, name=nm)

        wt_all = ptile([128, NPAIR, 2, V], FP8, "wt_all")
        wx_all = ptile([128, BPC, NPAIR, 2, RP], FP8, "wx_all")
        m2t = ptile([BPC, 2, NTOK], F32, "m2t")

        with (
            tc.tile_pool(name="hp", bufs=16) as hp_pool,
            tc.tile_pool(name="scr", bufs=2) as scr_pool,
            tc.tile_pool(name="small", bufs=4) as small_pool,
            tc.tile_pool(name="pt", bufs=2) as pt_pool,
            tc.tile_pool(name="ps", bufs=3, space="PSUM") as ps_pool,
            tc.tile_pool(name="gl", bufs=2, space="PSUM") as gl_pool,
        ):
            # ---- DMA issue order is the phase-1 critical path: chunk-0
            # activations and the first two v-slices of W go first so the
            # first matmul isn't gated on the full 8 MB of weights.
            hp_c0 = []
            for bb in range(BPC):
                hp_t = hp_pool.tile([128, NPAIR, 2, TCP], FP8, tag="hp",
                                    name="hp_t")
                nc.sync.dma_start(hp_t[:], hpt_d[bb, 0])
                hp_c0.append(hp_t)
            for pe in range(NPAIR):
                for i in range(2):
                    nc.sync.dma_start(wt_all[:, pe, i, 0:2 * VC],
                                      wtt_d[pe, i, :, 0:2 * VC])
            for bb in range(BPC):
                for pe in range(NPAIR):
                    for i in range(2):
                        nc.sync.dma_start(wx_all[:, bb, pe, i, 0:NTOK],
                                          wxt_d[bb, pe, i])
            nc.sync.dma_start(m2t[:], m2_d[:])
            for v in range(2, NV, 2):
                for pe in range(NPAIR):
                    for i in range(2):
                        nc.sync.dma_start(
                            wt_all[:, pe, i, v * VC:(v + 2) * VC],
                            wtt_d[pe, i, :, v * VC:(v + 2) * VC])

            ones125 = ptile([125, 1], BF16, "ones125")
            nc.vector.memset(ones125[:], 1.0)
            lnls_all = ptile([125, BPC * NCHUNK], BF16, "lnls_all")
            llacc = ptile([1, BPC], F32, "llacc")

            # ---- sweep state ----
            psweep = ptile([BPC, NTOK, T], BF16, "psweep")
            frow = ptile([BPC, T + 1], BF16, "frow")   # col k = F(t=k-1)
            orow = ptile([BPC, 2, T], BF16, "orow")    # col k = O(t=k)
            nc.vector.memset(frow[:], 0.0)
            nc.vector.memset(frow[:, 0:1], 1.0)        # F_0(-1) = B_0(-1) = 1
            nc.vector.memset(orow[:], 0.0)
            zrow = ptile([BPC, T], BF16, "zrow")
            nc.vector.memset(zrow[:], 0.0)
            vtmp = ptile([BPC, T], BF16, "vtmp")
            vtmp2 = ptile([BPC, T], BF16, "vtmp2")
            fend = ptile([BPC, NTOK], F32, "fend")   # F_j(TB-1)
            oend = ptile([BPC, NTOK], F32, "oend")   # O_j(TB-1)

            # ================= Phase 1: matmuls / lse / p~ =================
            for c in range(NCHUNK):
                t0 = c * TC
                for bb in range(BPC):
                    if c == 0:
                        hp_t = hp_c0[bb]
                    else:
                        hp_t = hp_pool.tile([128, NPAIR, 2, TCP], FP8,
                                            tag="hp", name="hp_t")
                        nc.sync.dma_start(hp_t[:], hpt_d[bb, c])

                    spart = small_pool.tile([TC, NV // 2], F32, tag="spart",
                                            name="spart")
                    for k in range(NV // 2):
                        # two v-chunks -> two PSUM banks, one paired Act op
                        ps = ps_pool.tile([TC, 2, 512], F32, tag="ps",
                                          name="ps")
                        for h in range(2):
                            v = 2 * k + h
                            for pe in range(NPAIR):
                                nc.tensor.matmul(
                                    ps[:, h, 0:VC],
                                    hp_t[:, pe, :, 0:TC],
                                    wt_all[:, pe, :, v * VC:(v + 1) * VC],
                                    start=(pe == 0), stop=(pe == NPAIR - 1),
                                    perf_mode=DR)
                        scr = scr_pool.tile([TC, 2, VC], BF16, tag="scr",
                                            name="scr")
                        nc.scalar.activation(scr[:], ps[:, :, 0:VC], AF.Exp,
                                             bias=-C_SHIFT, scale=1.0,
                                             accum_out=spart[:, k:k + 1])
                    scr10 = small_pool.tile([TC, NV // 2], BF16, tag="scr10",
                                            name="scr10")
                    lsum = small_pool.tile([TC, 1], F32, tag="lsum",
                                           name="lsum")
                    nc.scalar.activation(scr10[:], spart[:], AF.Identity,
                                         accum_out=lsum[:])
                    col = bb * NCHUNK + c
                    nc.scalar.activation(lnls_all[:, col:col + 1], lsum[:],
                                         AF.Ln)

                    # glog^T [token-row, t], then p~ = exp(glog + D) in bf16
                    gl = gl_pool.tile([NTOK, TC], F32, tag="gl", name="gl")
                    for pe in range(NPAIR):
                        nc.tensor.matmul(
                            gl[:], wx_all[:, bb, pe, :, 0:NTOK],
                            hp_t[:, pe, :, 0:TC],
                            start=(pe == 0), stop=(pe == NPAIR - 1),
                            perf_mode=DR)
                    ptc = pt_pool.tile([NTOK, TC], BF16, tag="ptc",
                                       name="ptc")
                    nc.scalar.activation(ptc[:], gl[:], AF.Exp,
                                         bias=D_SHIFT, scale=1.0)
                    nc.sync.dma_start(psweep[bb:bb + 1, :, t0:t0 + TC],
                                      ptc[:])

            # ================= Phase 2: two-pass F/O scan sweep =============
            # Pass 1 covers frames [0, TB); its scans depend only on the
            # psweep columns of chunks 0..TB/TC-1, so Tile's subtile deps
            # let it run concurrently with the remaining phase-1 chunks.
            # Boundary state F_j(TB-1)/O_j(TB-1) carries into pass 2.
            TB = 2 * TC
            masked = set(masked_j)
            qrow = psweep[:, 0, :]          # blank p~ row
            # The scans' own cross-engine deps on the psweep DMAs are not
            # reliably tracked; DVE is in-order, so one gate copy per pass
            # whose read spans that pass's chunks provides the needed
            # happens-after for every scan behind it.
            nc.vector.tensor_copy(vtmp2[:, 0:TB], psweep[:, 0, 0:TB])
            for j in range(L + 1):
                loF = max(0, j - 1)
                if j == 0:
                    nc.vector.tensor_tensor_scan(
                        frow[:, 1:TB + 1], qrow[:, 0:TB],
                        zrow[:, 0:TB], 1.0, ALU.mult, ALU.add)
                else:
                    oprev = orow[:, (j - 1) % 2, :]
                    if j in masked:
                        # d1 = m2*O_{j-1}(t) + (1-m2)*q(t)*O_{j-1}(t-1)
                        nc.vector.memset(vtmp[:, loF:loF + 1], 0.0)
                        nc.vector.tensor_mul(vtmp[:, loF + 1:TB],
                                             qrow[:, loF + 1:TB],
                                             oprev[:, loF:TB - 1])
                        nc.vector.tensor_scalar_mul(vtmp[:, loF:TB],
                                                    vtmp[:, loF:TB],
                                                    m2t[:, 1, j:j + 1])
                        nc.vector.tensor_scalar_mul(vtmp2[:, loF:TB],
                                                    oprev[:, loF:TB],
                                                    m2t[:, 0, j:j + 1])
                        nc.vector.tensor_add(vtmp[:, loF:TB],
                                             vtmp[:, loF:TB],
                                             vtmp2[:, loF:TB])
                        d1 = vtmp[:, loF:TB]
                    else:
                        d1 = oprev[:, loF:TB]
                    nc.vector.tensor_tensor_scan(
                        frow[:, loF + 1:TB + 1], qrow[:, loF:TB],
                        d1, 0.0, ALU.mult, ALU.add)
                nc.vector.tensor_copy(fend[:, j:j + 1], frow[:, TB:TB + 1])
                if j == L:
                    break
                nc.vector.tensor_tensor_scan(
                    orow[:, j % 2, j:TB], frow[:, j:TB],
                    psweep[:, 1 + j, j:TB], 0.0, ALU.add, ALU.mult)
                nc.vector.tensor_copy(oend[:, j:j + 1],
                                      orow[:, j % 2, TB - 1:TB])

            # ---- pass 2: frames [TB, T) ----
            nc.vector.tensor_copy(vtmp2[:, TB:T], psweep[:, 0, TB:T])
            for j in range(L + 1):
                hiF = _hi(2 * j)
                # restore the frow halo col (F_j(TB-1)) clobbered by later js
                nc.vector.tensor_copy(frow[:, TB:TB + 1], fend[:, j:j + 1])
                if j == 0:
                    d1 = zrow[:, TB:hiF + 1]
                else:
                    oprev = orow[:, (j - 1) % 2, :]
                    if j in masked:
                        nc.vector.tensor_mul(vtmp[:, TB:TB + 1],
                                             qrow[:, TB:TB + 1],
                                             oend[:, j - 1:j])
                        nc.vector.tensor_mul(vtmp[:, TB + 1:hiF + 1],
                                             qrow[:, TB + 1:hiF + 1],
                                             oprev[:, TB:hiF])
                        nc.vector.tensor_scalar_mul(vtmp[:, TB:hiF + 1],
                                                    vtmp[:, TB:hiF + 1],
                                                    m2t[:, 1, j:j + 1])
                        nc.vector.tensor_scalar_mul(vtmp2[:, TB:hiF + 1],
                                                    oprev[:, TB:hiF + 1],
                                                    m2t[:, 0, j:j + 1])
                        nc.vector.tensor_add(vtmp[:, TB:hiF + 1],
                                             vtmp[:, TB:hiF + 1],
                                             vtmp2[:, TB:hiF + 1])
                        d1 = vtmp[:, TB:hiF + 1]
                    else:
                        d1 = oprev[:, TB:hiF + 1]
                nc.vector.tensor_tensor_scan(
                    frow[:, TB + 1:hiF + 2], qrow[:, TB:hiF + 1],
                    d1, fend[:, j:j + 1], ALU.mult, ALU.add)
                if j == L:
                    break
                hiO = _hi(2 * j + 1)
                nc.vector.tensor_tensor_scan(
                    orow[:, j % 2, TB:hiO + 1], frow[:, TB:hiO + 1],
                    psweep[:, 1 + j, TB:hiO + 1], oend[:, j:j + 1],
                    ALU.add, ALU.mult)

            # ================= finalize: ll = ln F_L(T-1) - llacc ==========
            # Sum_t ln lsum: ones-matmul partition reduce (PE is idle now),
            # reusing a ps-pool PSUM slot; columns are bb-major.
            psl = ps_pool.tile([TC, 2, 512], F32, tag="ps", name="psl")
            nc.tensor.matmul(psl[0:1, 0, 0:BPC * NCHUNK], ones125[:],
                             lnls_all[:], start=True, stop=True)
            for bb in range(BPC):
                nc.vector.tensor_reduce(
                    llacc[:, bb:bb + 1],
                    psl[0:1, 0, bb * NCHUNK:(bb + 1) * NCHUNK],
                    axis=AX.X, op=ALU.add)
            lnu = ptile([BPC, 1], F32, "lnu")
            nc.scalar.activation(lnu[:], frow[:, T:T + 1], AF.Ln)
            llf = ptile([1, BPC], F32, "llf")
            nc.sync.dma_start(llf[:], lnu[:])   # [4,1] -> [1,4]
            dif = ptile([1, BPC], F32, "dif")
            nc.vector.tensor_tensor(dif[:], llf[:], llacc[:], ALU.subtract)
            tot = ptile([1, 1], F32, "tot")
            nc.vector.tensor_reduce(tot[:], dif[:], axis=AX.X, op=ALU.add)
            nc.sync.dma_start(out_d[:], tot[:])

    nc.compile()
    return nc


def prep_in_maps(hpad, W, b, ys):
    """Host-side layout prep shared by kernel() and test harnesses."""
    f8 = mybir.dt.np(FP8)
    W = np.asarray(W)
    ys = np.asarray(ys)
    # allow skip into label j (j >= 1): labels differ; j=0 has no skip source
    allow = np.ones((B, NTOK), np.float32)
    allow[:, 1:L] = (ys[:, 1:] != ys[:, :-1]).astype(np.float32)
    masked_j = tuple(sorted(
        j for j in range(1, L) if not allow[:, j].all()))
    m2 = np.stack([allow, 1.0 - allow], axis=1)       # [B, 2, NTOK]

    hpT = np.ascontiguousarray(hpad.transpose(0, 2, 1)).astype(f8)
    hpT = hpT.reshape(B, NPAIR, 2, 128, NCHUNK, TC)
    hpP = np.zeros((B, NCHUNK, 128, NPAIR, 2, TCP), dtype=f8)
    hpP[..., :TC] = hpT.transpose(0, 4, 3, 1, 2, 5)
    hpT = hpP.reshape(B, NCHUNK, 128, NPAIR * 2 * TCP)
    wtT = np.ascontiguousarray(W.T).astype(f8).reshape(NPAIR, 2, 128, V)
    # distinct tokens per example: row 0 = blank, row 1+j = label j
    toks = np.concatenate([np.zeros((B, 1), np.int64),
                           ys.astype(np.int64)], axis=1)   # [B, NTOK]
    wext = np.ascontiguousarray(
        W[toks.reshape(-1)].reshape(B, NTOK, E).transpose(0, 2, 1)
    ).astype(f8).reshape(B, NPAIR, 2, 128, NTOK)

    in_maps = []
    for c in range(NCORE):
        sl = slice(c * BPC, (c + 1) * BPC)
        in_maps.append({
            "hpt": np.ascontiguousarray(hpT[sl]),
            "wtt": wtT,
            "wxt": np.ascontiguousarray(wext[sl]),
            "m2": np.ascontiguousarray(m2[sl]),
        })
    return in_maps, masked_j


def kernel(hpad, W, b, ys):
    assert hpad.shape == (B, T, E) and W.shape == (V, E) and ys.shape == (B, L)
    assert not np.any(np.asarray(b)), "kernel assumes b == 0 (per problem spec)"

    in_maps, masked_j = prep_in_maps(hpad, W, b, ys)
    key = ("nc", masked_j)
    if key not in _cache:
        _cache[key] = _build_nc(masked_j)
    nc = _cache[key]
    _cache["nc_last"] = (nc, in_maps)

    res = run_bass_kernel_spmd(nc, in_maps, core_ids=list(range(NCORE)))
    tot = sum(float(r["out"][0, 0]) for r in res.results)
    ll_sum = tot - B * T * (C_SHIFT + D_SHIFT)
    return np.float32(-ll_sum / B)


# revision 20
# speedup vs baseline: 1.0011x; 1.0011x over previous
"""CTC loss (nn_CTC_28819230556189) on 8 Trainium2 NeuronCores via Bass/Tile.

Data-parallel over batch (4 examples/core). Per core:

  Phase 1 (PE + Act):  logits = hpad @ W.T in fp8 DoubleRow (K=256/matmul);
    lse side:  exp(logit - C) accumulated over V -> lsum[t]; ln(lsum) columns
               collected and partition-reduced ONCE at the end via a
               ones-matmul -> Sum_t ln lsum per example (llacc).
    glog side: the extended label sequence has only 101 distinct tokens per
               example (blank + 100 labels), so glog^T is computed as a
               [101, t] matmul; p~ = exp(glog + D) in bf16 is DMA-transposed
               into psweep[example, row, t] (row 0 = blank, row 1+j = label j).
    The -lse term is NOT folded into p~: every CTC path takes exactly one
    emission per frame, so ll = ln(sum B~) - Sum_t lse_t - T*D, with
    lse_t = ln lsum_t + C.

  Phase 2 (DVE): CTC forward DP as a label-sweep of pure affine scans
    (tensor_tensor_scan, fp32 internal state).  With E_j = alpha[blank 2j],
    O_j = alpha[label 2j+1], and F_j = E_j + allow_j * O_{j-1}:
        F_j(t) = q(t)*F_j(t-1) + O_{j-1}(t)        (scan: mult, add)
        O_j(t) = (O_j(t-1) + F_j(t-1)) * p_j(t)    (scan: add, mult)
    and the likelihood is simply F_L(T-1) = alpha_T[S-1] + alpha_T[S-2].
    201 scans total, no elementwise adds; each scan runs only over the
    frame window from which terminal states remain reachable (~402 of 500).
    Repeated labels (skip disallowed) get a rare blended-input fixup.

  Loss partials summed on host (no collectives needed).
"""

import numpy as np

import concourse.bass as bass
import concourse.bacc as bacc
import concourse.tile as tile
import concourse.mybir as mybir
from concourse.bass_utils import run_bass_kernel_spmd

BF16 = mybir.dt.bfloat16
F32 = mybir.dt.float32
FP8 = mybir.dt.float8e4
AF = mybir.ActivationFunctionType
ALU = mybir.AluOpType
AX = mybir.AxisListType
DR = mybir.MatmulPerfMode.DoubleRow

# Problem shapes (hardcoded per spec nn_CTC_28819230556189)
B, T, E, V, L = 32, 500, 1024, 5000, 100
S = 2 * L + 1           # 201 extended labels
NCORE = 8
BPC = B // NCORE        # 4 examples per core
NPAIR = E // 256        # 4 double-row K-pairs (256 contraction each)
TC = 125                # time chunk
NCHUNK = T // TC        # 4
VC = 500                # v-chunk width (one PSUM bank in f32)
NV = V // VC            # 10
C_SHIFT = 4.0           # logsumexp constant shift (logits ~ N(0,1))
D_SHIFT = -1.1          # p~ = exp(glog + D); keeps ln(sum B~) drift ~ 0
NTOK = L + 1            # distinct p~ rows per example (blank + labels)
TCP = 128               # hp stationary pair-stride (16B-aligned pad of TC)
RP = 112                # wext stationary pair-stride (16B-aligned pad of NTOK)

_cache = {}


def _hi(s):
    """Last frame (inclusive) from which extended state s can still reach a
    terminal state ({S-2, S-1}) by frame T-1."""
    need = max(0, (S - 2) - s)
    return min(T - 1, T - 1 - (need + 1) // 2)


def _build_nc(masked_j):
    """masked_j: sorted tuple of label indices j (1..L-1) where some example
    has ys[j] == ys[j-1] (skip transition disallowed) -> those F_j scans get
    a per-example blended input; all other j use O_{j-1} directly."""
    nc = bacc.Bacc("TRN2", target_bir_lowering=False, debug=False,
                   enable_asserts=False)

    for val in (-C_SHIFT, D_SHIFT):
        cth = nc.alloc_sbuf_tensor(f"const-f32-{val}", [128, 1], F32)
        nc.gpsimd.memset(cth.ap(), val)
        nc.const_aps.aps[(F32, val)] = cth.ap()
    nc.all_engine_barrier()

    hpt_d = nc.dram_tensor("hpt", [BPC, NCHUNK, 128, NPAIR * 2 * TCP],
                           FP8, kind="ExternalInput")
    wtt_d = nc.dram_tensor("wtt", [NPAIR, 2, 128, V], FP8,
                           kind="ExternalInput")
    wxt_d = nc.dram_tensor("wxt", [BPC, NPAIR, 2, 128, NTOK], FP8,
                           kind="ExternalInput")
    # m2[:, 0, j] = allow skip into label j; m2[:, 1, j] = 1 - that
    m2_d = nc.dram_tensor("m2", [BPC, 2, NTOK], F32, kind="ExternalInput")
    out_d = nc.dram_tensor("out", [1, 1], F32, kind="ExternalOutput")

    with tile.TileContext(nc) as tc:
      with tc.tile_pool(name="persist", bufs=1) as pers:
        def ptile(shape, dtype, nm):
            return pers.tile(shape, dtype, tag=nm, name=nm)

        wt_all = ptile([128, NPAIR, 2, V], FP8, "wt_all")
        wx_all = ptile([128, BPC, NPAIR, 2, RP], FP8, "wx_all")
        m2t = ptile([BPC, 2, NTOK], F32, "m2t")

        with (
            tc.tile_pool(name="hp", bufs=16) as hp_pool,
            tc.tile_pool(name="scr", bufs=2) as scr_pool,
            tc.tile_pool(name="small", bufs=4) as small_pool,
            tc.tile_pool(name="pt", bufs=2) as pt_pool,
            tc.tile_pool(name="ps", bufs=3, space="PSUM") as ps_pool,
            tc.tile_pool(name="gl", bufs=2, space="PSUM") as gl_pool,
        ):
            # ---- DMA issue order is the phase-1 critical path: chunk-0
            # activations and the first two v-slices of W go first so the
            # first matmul isn't gated on the full 8 MB of weights.
            hp_c0 = []
            for bb in range(BPC):
                hp_t = hp_pool.tile([128, NPAIR, 2, TCP], FP8, tag="hp",
                                    name="hp_t")
                nc.sync.dma_start(hp_t[:], hpt_d[bb, 0])
                hp_c0.append(hp_t)
            for pe in range(NPAIR):
                for i in range(2):
                    nc.sync.dma_start(wt_all[:, pe, i, 0:2 * VC],
                                      wtt_d[pe, i, :, 0:2 * VC])
            for bb in range(BPC):
                for pe in range(NPAIR):
                    for i in range(2):
                        nc.sync.dma_start(wx_all[:, bb, pe, i, 0:NTOK],
                                          wxt_d[bb, pe, i])
            nc.sync.dma_start(m2t[:], m2_d[:])
            for v in range(2, NV, 2):
                for pe in range(NPAIR):
                    for i in range(2):
                        nc.sync.dma_start(
                            wt_all[:, pe, i, v * VC:(v + 2) * VC],
                            wtt_d[pe, i, :, v * VC:(v + 2) * VC])

            ones125 = ptile([125, 1], BF16, "ones125")
            nc.vector.memset(ones125[:], 1.0)
            lnls_all = ptile([125, BPC * NCHUNK], BF16, "lnls_all")
            llacc = ptile([1, BPC], F32, "llacc")

            # ---- sweep state ----
            # psweep split at the pass boundary into two physical tiles so
            # pass-1 scans only depend (even at whole-tile granularity) on
            # the chunk-0/1 DMAs and can overlap the rest of phase 1.
            psA = ptile([BPC, NTOK, 2 * TC], BF16, "psA")
            psB = ptile([BPC, NTOK, T - 2 * TC], BF16, "psB")
            frow = ptile([BPC, T + 1], BF16, "frow")   # col k = F(t=k-1)
            orow = ptile([BPC, 2, T], BF16, "orow")    # col k = O(t=k)
            nc.vector.memset(frow[:], 0.0)
            nc.vector.memset(frow[:, 0:1], 1.0)        # F_0(-1) = B_0(-1) = 1
            nc.vector.memset(orow[:], 0.0)
            zrow = ptile([BPC, T], BF16, "zrow")
            nc.vector.memset(zrow[:], 0.0)
            vtmp = ptile([BPC, T], BF16, "vtmp")
            vtmp2 = ptile([BPC, T], BF16, "vtmp2")
            fend = ptile([BPC, NTOK], F32, "fend")   # F_j(TB-1)
            oend = ptile([BPC, NTOK], F32, "oend")   # O_j(TB-1)

            # ================= Phase 1: matmuls / lse / p~ =================
            for c in range(NCHUNK):
                t0 = c * TC
                for bb in range(BPC):
                    if c == 0:
                        hp_t = hp_c0[bb]
                    else:
                        hp_t = hp_pool.tile([128, NPAIR, 2, TCP], FP8,
                                            tag="hp", name="hp_t")
                        nc.sync.dma_start(hp_t[:], hpt_d[bb, c])

                    spart = small_pool.tile([TC, NV // 2], F32, tag="spart",
                                            name="spart")
                    for k in range(NV // 2):
                        # two v-chunks -> two PSUM banks, one paired Act op
                        ps = ps_pool.tile([TC, 2, 512], F32, tag="ps",
                                          name="ps")
                        for h in range(2):
                            v = 2 * k + h
                            for pe in range(NPAIR):
                                nc.tensor.matmul(
                                    ps[:, h, 0:VC],
                                    hp_t[:, pe, :, 0:TC],
                                    wt_all[:, pe, :, v * VC:(v + 1) * VC],
                                    start=(pe == 0), stop=(pe == NPAIR - 1),
                                    perf_mode=DR)
                        scr = scr_pool.tile([TC, 2, VC], BF16, tag="scr",
                                            name="scr")
                        nc.scalar.activation(scr[:], ps[:, :, 0:VC], AF.Exp,
                                             bias=-C_SHIFT, scale=1.0,
                                             accum_out=spart[:, k:k + 1])
                    scr10 = small_pool.tile([TC, NV // 2], BF16, tag="scr10",
                                            name="scr10")
                    lsum = small_pool.tile([TC, 1], F32, tag="lsum",
                                           name="lsum")
                    nc.scalar.activation(scr10[:], spart[:], AF.Identity,
                                         accum_out=lsum[:])
                    col = bb * NCHUNK + c
                    nc.scalar.activation(lnls_all[:, col:col + 1], lsum[:],
                                         AF.Ln)

                    # glog^T [token-row, t], then p~ = exp(glog + D) in bf16
                    gl = gl_pool.tile([NTOK, TC], F32, tag="gl", name="gl")
                    for pe in range(NPAIR):
                        nc.tensor.matmul(
                            gl[:], wx_all[:, bb, pe, :, 0:NTOK],
                            hp_t[:, pe, :, 0:TC],
                            start=(pe == 0), stop=(pe == NPAIR - 1),
                            perf_mode=DR)
                    ptc = pt_pool.tile([NTOK, TC], BF16, tag="ptc",
                                       name="ptc")
                    nc.scalar.activation(ptc[:], gl[:], AF.Exp,
                                         bias=D_SHIFT, scale=1.0)
                    if c < 2:
                        nc.sync.dma_start(psA[bb:bb + 1, :, t0:t0 + TC],
                                          ptc[:])
                    else:
                        nc.sync.dma_start(
                            psB[bb:bb + 1, :, t0 - 2 * TC:t0 - 2 * TC + TC],
                            ptc[:])

            # ================= Phase 2: two-pass F/O scan sweep =============
            # Pass 1 covers frames [0, TB); its scans depend only on the
            # psweep columns of chunks 0..TB/TC-1, so Tile's subtile deps
            # let it run concurrently with the remaining phase-1 chunks.
            # Boundary state F_j(TB-1)/O_j(TB-1) carries into pass 2.
            TB = 2 * TC
            masked = set(masked_j)
            qrow = psA[:, 0, :]             # blank p~ row, frames [0, TB)
            # The scans' own cross-engine deps on the psweep DMAs are not
            # reliably tracked; DVE is in-order, so one gate copy per pass
            # whose read spans that pass's tile provides the needed
            # happens-after for every scan behind it.
            nc.vector.tensor_copy(vtmp2[:, 0:TB], psA[:, 0, :])
            for j in range(L + 1):
                loF = max(0, j - 1)
                if j == 0:
                    nc.vector.tensor_tensor_scan(
                        frow[:, 1:TB + 1], qrow[:, 0:TB],
                        zrow[:, 0:TB], 1.0, ALU.mult, ALU.add)
                else:
                    oprev = orow[:, (j - 1) % 2, :]
                    if j in masked:
                        # d1 = m2*O_{j-1}(t) + (1-m2)*q(t)*O_{j-1}(t-1)
                        nc.vector.memset(vtmp[:, loF:loF + 1], 0.0)
                        nc.vector.tensor_mul(vtmp[:, loF + 1:TB],
                                             qrow[:, loF + 1:TB],
                                             oprev[:, loF:TB - 1])
                        nc.vector.tensor_scalar_mul(vtmp[:, loF:TB],
                                                    vtmp[:, loF:TB],
                                                    m2t[:, 1, j:j + 1])
                        nc.vector.tensor_scalar_mul(vtmp2[:, loF:TB],
                                                    oprev[:, loF:TB],
                                                    m2t[:, 0, j:j + 1])
                        nc.vector.tensor_add(vtmp[:, loF:TB],
                                             vtmp[:, loF:TB],
                                             vtmp2[:, loF:TB])
                        d1 = vtmp[:, loF:TB]
                    else:
                        d1 = oprev[:, loF:TB]
                    nc.vector.tensor_tensor_scan(
                        frow[:, loF + 1:TB + 1], qrow[:, loF:TB],
                        d1, 0.0, ALU.mult, ALU.add)
                nc.vector.tensor_copy(fend[:, j:j + 1], frow[:, TB:TB + 1])
                if j == L:
                    break
                nc.vector.tensor_tensor_scan(
                    orow[:, j % 2, j:TB], frow[:, j:TB],
                    psA[:, 1 + j, j:TB], 0.0, ALU.add, ALU.mult)
                nc.vector.tensor_copy(oend[:, j:j + 1],
                                      orow[:, j % 2, TB - 1:TB])

            # ---- pass 2: frames [TB, T) ----
            nc.vector.tensor_copy(vtmp2[:, TB:T], psB[:, 0, :])
            qrowB = psB[:, 0, :]
            for j in range(L + 1):
                hiF = _hi(2 * j)
                # restore the frow halo col (F_j(TB-1)) clobbered by later js
                nc.vector.tensor_copy(frow[:, TB:TB + 1], fend[:, j:j + 1])
                if j == 0:
                    d1 = zrow[:, TB:hiF + 1]
                else:
                    oprev = orow[:, (j - 1) % 2, :]
                    if j in masked:
                        nc.vector.tensor_mul(vtmp[:, TB:TB + 1],
                                             qrowB[:, 0:1],
                                             oend[:, j - 1:j])
                        nc.vector.tensor_mul(vtmp[:, TB + 1:hiF + 1],
                                             qrowB[:, 1:hiF + 1 - TB],
                                             oprev[:, TB:hiF])
                        nc.vector.tensor_scalar_mul(vtmp[:, TB:hiF + 1],
                                                    vtmp[:, TB:hiF + 1],
                                                    m2t[:, 1, j:j + 1])
                        nc.vector.tensor_scalar_mul(vtmp2[:, TB:hiF + 1],
                                                    oprev[:, TB:hiF + 1],
                                                    m2t[:, 0, j:j + 1])
                        nc.vector.tensor_add(vtmp[:, TB:hiF + 1],
                                             vtmp[:, TB:hiF + 1],
                                             vtmp2[:, TB:hiF + 1])
                        d1 = vtmp[:, TB:hiF + 1]
                    else:
                        d1 = oprev[:, TB:hiF + 1]
                nc.vector.tensor_tensor_scan(
                    frow[:, TB + 1:hiF + 2], qrowB[:, 0:hiF + 1 - TB],
                    d1, fend[:, j:j + 1], ALU.mult, ALU.add)
                if j == L:
                    break
                hiO = _hi(2 * j + 1)
                nc.vector.tensor_tensor_scan(
                    orow[:, j % 2, TB:hiO + 1], frow[:, TB:hiO + 1],
                    psB[:, 1 + j, 0:hiO + 1 - TB], oend[:, j:j + 1],
                    ALU.add, ALU.mult)

            # ================= finalize: ll = ln F_L(T-1) - llacc ==========
            # Sum_t ln lsum: ones-matmul partition reduce (PE is idle now),
            # reusing a ps-pool PSUM slot; columns are bb-major.
            psl = ps_pool.tile([TC, 2, 512], F32, tag="ps", name="psl")
            nc.tensor.matmul(psl[0:1, 0, 0:BPC * NCHUNK], ones125[:],
                             lnls_all[:], start=True, stop=True)
            for bb in range(BPC):
                nc.vector.tensor_reduce(
                    llacc[:, bb:bb + 1],
                    psl[0:1, 0, bb * NCHUNK:(bb + 1) * NCHUNK],
                    axis=AX.X, op=ALU.add)
            lnu = ptile([BPC, 1], F32, "lnu")
            nc.scalar.activation(lnu[:], frow[:, T:T + 1], AF.Ln)
            llf = ptile([1, BPC], F32, "llf")
            nc.sync.dma_start(llf[:], lnu[:])   # [4,1] -> [1,4]
            dif = ptile([1, BPC], F32, "dif")
            nc.vector.tensor_tensor(dif[:], llf[:], llacc[:], ALU.subtract)
            tot = ptile([1, 1], F32, "tot")
            nc.vector.tensor_reduce(tot[:], dif[:], axis=AX.X, op=ALU.add)
            nc.sync.dma_start(out_d[:], tot[:])

    nc.compile()
    return nc


def prep_in_maps(hpad, W, b, ys):
    """Host-side layout prep shared by kernel() and test harnesses."""
    f8 = mybir.dt.np(FP8)
    W = np.asarray(W)
    ys = np.asarray(ys)
    # allow skip into label j (j >= 1): labels differ; j=0 has no skip source
    allow = np.ones((B, NTOK), np.float32)
    allow[:, 1:L] = (ys[:, 1:] != ys[:, :-1]).astype(np.float32)
    masked_j = tuple(sorted(
        j for j in range(1, L) if not allow[:, j].all()))
    m2 = np.stack([allow, 1.0 - allow], axis=1)       # [B, 2, NTOK]

    hpT = np.ascontiguousarray(hpad.transpose(0, 2, 1)).astype(f8)
    hpT = hpT.reshape(B, NPAIR, 2, 128, NCHUNK, TC)
    hpP = np.zeros((B, NCHUNK, 128, NPAIR, 2, TCP), dtype=f8)
    hpP[..., :TC] = hpT.transpose(0, 4, 3, 1, 2, 5)
    hpT = hpP.reshape(B, NCHUNK, 128, NPAIR * 2 * TCP)
    wtT = np.ascontiguousarray(W.T).astype(f8).reshape(NPAIR, 2, 128, V)
    # distinct tokens per example: row 0 = blank, row 1+j = label j
    toks = np.concatenate([np.zeros((B, 1), np.int64),
                           ys.astype(np.int64)], axis=1)   # [B, NTOK]
    wext = np.ascontiguousarray(
        W[toks.reshape(-1)].reshape(B, NTOK, E).transpose(0, 2, 1)
    ).astype(f8).reshape(B, NPAIR, 2, 128, NTOK)

    in_maps = []
    for c in range(NCORE):
        sl = slice(c * BPC, (c + 1) * BPC)
        in_maps.append({
            "hpt": np.ascontiguousarray(hpT[sl]),
            "wtt": wtT,
            "wxt": np.ascontiguousarray(wext[sl]),
            "m2": np.ascontiguousarray(m2[sl]),
        })
    return in_maps, masked_j


def kernel(hpad, W, b, ys):
    assert hpad.shape == (B, T, E) and W.shape == (V, E) and ys.shape == (B, L)
    assert not np.any(np.asarray(b)), "kernel assumes b == 0 (per problem spec)"

    in_maps, masked_j = prep_in_maps(hpad, W, b, ys)
    key = ("nc", masked_j)
    if key not in _cache:
        _cache[key] = _build_nc(masked_j)
    nc = _cache[key]
    _cache["nc_last"] = (nc, in_maps)

    res = run_bass_kernel_spmd(nc, in_maps, core_ids=list(range(NCORE)))
    tot = sum(float(r["out"][0, 0]) for r in res.results)
    ll_sum = tot - B * T * (C_SHIFT + D_SHIFT)
    return np.float32(-ll_sum / B)
